# revision 2
# baseline (speedup 1.0000x reference)
"""3-layer GAT on 8 Trainium2 NeuronCores.

Strategy: destination-node sharding. Edges (+self-loops) are sorted by dst and
partitioned into 8 shards of 2500 dst nodes; each shard is split into windows
of 128 dst nodes. Node features live in a packed per-node gather table:
[h as fp8 | a_src-logit, a_dst-logit as bf16] in a 512B (256B for layer 2)
row. Per window, one dma_gather fetches the rows of all edge sources plus the
window's own 128 dst rows (for the dst logits). Per-edge dst logits are
reconstructed with a transposed one-hot mask matmul, attention weights
ex=exp(lrelu(al_s+al_d)) are folded into the gathered features, and
segment-softmax + aggregation run as one-hot matmuls (host-precomputed fp8
masks) accumulating in PSUM: U = sum_e ex*h[src], s = sum_e ex,
out = U * recip(s). Per-node tables come from a dense matmul
act @ [W | W@as | W@ad] on every core; activations cross layers via two
half-sized AllGathers (overlapped with the edge phase) of transposed shards.
"""
import sys

for _p in ("/opt/trn_rl_repo",):
    if _p not in sys.path:
        sys.path.insert(0, _p)

import ml_dtypes
import numpy as np

import concourse.bacc as bacc
import concourse.bass as bass
import concourse.mybir as mybir
import concourse.tile as tile
from concourse import bass_utils
from concourse.library_config import mlp

F32 = mybir.dt.float32
BF16 = mybir.dt.bfloat16
FP8 = mybir.dt.float8e4
I16 = mybir.dt.int16
ALU = mybir.AluOpType
ACTF = mybir.ActivationFunctionType
BF = ml_dtypes.bfloat16
F8 = ml_dtypes.float8_e4m3

NEG_SLOPE = 0.2
EPS = 1e-16
# gather chunk (tiles of 128 idxs) per dma_gather; chunk descriptors must
# fit the SWDGE descriptor ring (dynamic_dma_scratch_size/16) or the device
# hangs
GCH = 8
DMA_SCRATCH = 16384
NSWQ = 1            # >1 SWDGE queue breaks completion semantics on HW
NCH = 5             # inter-layer AllGather chunks (overlap granularity)
CC_OFF = 4          # iterations after a chunk's last s2 before its AllGather


class Cfg:
    def __init__(self, N=20000, IN=128, HID=64, HEADS=4, OUT=64, NC=8):
        assert N % NC == 0
        self.N, self.IN, self.HID, self.HEADS, self.OUT, self.NC = (
            N, IN, HID, HEADS, OUT, NC)
        self.NLOC = N // NC
        self.NWIN = -(-self.NLOC // 128)
        self.CW = self.NWIN // NCH          # windows per AllGather chunk
        self.NPAD = self.NWIN * 128
        self.NTOT = NC * self.NPAD
        self.F1 = HEADS * HID                      # 256
        # per-layer: (F_in, F_out, heads, row_bytes, relu)
        self.layers = [
            (IN, self.F1, HEADS, self._rw(self.F1, HEADS), True),
            (self.F1, self.F1, HEADS, self._rw(self.F1, HEADS), True),
            (self.F1, OUT, 1, self._rw(OUT, 1), False),
        ]

    @staticmethod
    def _rw(fout, heads):
        # packed row: fout fp8 bytes + 4*heads logit bytes, padded so the
        # dma_gather row size is a multiple of 256B
        need = fout + 4 * heads
        return ((need + 255) // 256) * 256


def _wrap16(idx_flat):
    """[M] -> [128, M//16] int16 index layout for gpsimd dma_gather."""
    a = np.asarray(idx_flat, np.int16).reshape(-1, 16).T
    return np.ascontiguousarray(np.tile(a, (8, 1)))


def host_prep(cfg, x, edge_index, weights):
    """The reference's appended self-loops are NOT placed in the edge
    stream: each window's dst-extra gather tile already holds every dst
    node's row, so the self-loop term is added straight from it (saving a
    gather tile per window). Natural (i,i) edges in edge_index stay."""
    N, NC, NLOC, NWIN = cfg.N, cfg.NC, cfg.NLOC, cfg.NWIN
    src = np.asarray(edge_index[0], np.int64)
    dst = np.asarray(edge_index[1], np.int64)
    order = np.argsort(dst, kind="stable")
    src, dst = src[order], dst[order]

    core_of = dst // NLOC
    wloc = (dst % NLOC) // 128
    dloc = (dst % NLOC) % 128

    counts = np.zeros((NC, NWIN), np.int64)
    np.add.at(counts, (core_of, wloc), 1)
    Tw = int(-(-counts.max() // 128))
    TG = Tw + 1                      # gathered tiles: Tw edge + 1 dst-row

    def row_of(v):
        return (v // NLOC) * cfg.NPAD + (v % NLOC)

    per_core = []
    ES = NWIN * Tw * 128             # edge slots (mask addressing)
    EI = NWIN * TG * 128             # gather index stream length
    for c in range(NC):
        # pad slots gather row 0 (skipping via negative indices starves
        # some SDMA engines of descriptors and hangs the device)
        gidx = np.zeros(EI, np.int64)
        sm = np.zeros((128, ES), np.uint8)
        smT = np.zeros((128, ES), np.uint8)
        m = core_of == c
        sc, wc, dlc = src[m], wloc[m], dloc[m]
        for w in range(NWIN):
            wm = wc == w
            n = int(wm.sum())
            gbase = w * TG * 128
            # tile 0: the window's own 128 dst rows (clipped to NLOC), then
            # the edge tiles with pads trailing. Slots are ordered by SRC so
            # the gather walks the table in ascending address order (HBM
            # locality); masks encode slot->dst so any slot order is valid.
            own = c * NLOC + w * 128 + np.arange(128)
            own = np.minimum(own, (c + 1) * NLOC - 1)
            gidx[gbase:gbase + 128] = row_of(own)
            so = np.argsort(sc[wm], kind="stable")
            gidx[gbase + 128:gbase + 128 + n] = row_of(sc[wm][so])
            # masks: slot j of window w -> tile t=j//128, lane e=j%128
            j = np.arange(n)
            t, e, d = j // 128, j % 128, dlc[wm][so]
            col = (w * Tw + t) * 128
            sm[e, col + d] = 0x38
            smT[d, col + e] = 0x38
        # interleave per window: [128, w, {sm,smT}, Tw*128] for single loads
        mm = np.stack([sm.reshape(128, NWIN, Tw * 128),
                       smT.reshape(128, NWIN, Tw * 128)], axis=2)
        per_core.append({
            "gidx": _wrap16(gidx),
            "masks": np.ascontiguousarray(
                mm.reshape(128, NWIN * 2 * Tw * 128).view(F8)),
        })

    xT = np.zeros((cfg.IN, cfg.NTOT), BF)
    xv = np.asarray(x, np.float32)
    for c in range(NC):
        xT[:, c * cfg.NPAD:c * cfg.NPAD + NLOC] = xv[c * NLOC:(c + 1) * NLOC].T

    def wcat(W, a_s, a_d, heads, hid):
        W = np.asarray(W, np.float32)
        a_s = np.asarray(a_s, np.float32).reshape(heads, hid)
        a_d = np.asarray(a_d, np.float32).reshape(heads, hid)
        was = np.stack([W[:, h * hid:(h + 1) * hid] @ a_s[h]
                        for h in range(heads)], axis=1)
        wad = np.stack([W[:, h * hid:(h + 1) * hid] @ a_d[h]
                        for h in range(heads)], axis=1)
        cat = np.concatenate([W, was, wad], axis=1)  # [F_in, NW2]
        KB = W.shape[0] // 128
        return np.concatenate(
            [cat[kb * 128:(kb + 1) * 128] for kb in range(KB)],
            axis=1).astype(BF)

    H, HID_, OUT_ = cfg.HEADS, cfg.HID, cfg.OUT
    shared = {
        "xT": xT,
        "wcat0": wcat(weights["W0"], weights["as0"], weights["ad0"], H, HID_),
        "wcat1": wcat(weights["W1"], weights["as1"], weights["ad1"], H, HID_),
        "wcat2": wcat(weights["W2"], weights["as2"], weights["ad2"], 1, OUT_),
        "ident": np.eye(128, dtype=np.float32),
        "b0r": np.tile(np.asarray(weights["b0"], np.float32)[None, :], (128, 1)),
        "b1r": np.tile(np.asarray(weights["b1"], np.float32)[None, :], (128, 1)),
        "b2r": np.tile(np.asarray(weights["b2"], np.float32)[None, :], (128, 1)),
    }
    return shared, per_core, Tw


ABLATE = set()       # timing experiments: {"cc", "edge", "dense", "gather"}


def build_module(cfg, Tw, repeat=1):
    abl = ABLATE
    nc = bacc.Bacc("TRN2", target_bir_lowering=False, debug=False,
                   num_devices=cfg.NC, dynamic_dma_scratch_size=DMA_SCRATCH,
                   num_swdge_queues=NSWQ)
    NWIN, NPAD, NTOT, NC = cfg.NWIN, cfg.NPAD, cfg.NTOT, cfg.NC
    CW = cfg.CW
    TG = Tw + 1
    ES = NWIN * Tw * 128
    EI = NWIN * TG * 128

    def din(name, shape, dtype=F32):
        return nc.dram_tensor(name, list(shape), dtype, kind="ExternalInput")

    xT = din("xT", (cfg.IN, NTOT), BF16)
    wc = [din("wcat0", (128, cfg.F1 + 2 * cfg.HEADS), BF16),
          din("wcat1", (128, 2 * (cfg.F1 + 2 * cfg.HEADS)), BF16),
          din("wcat2", (128, 2 * (cfg.OUT + 2)), BF16)]
    ident = din("ident", (128, 128))
    brep = [din("b0r", (128, cfg.F1)), din("b1r", (128, cfg.F1)),
            din("b2r", (128, cfg.OUT))]
    gidx_d = din("gidx", (128, EI // 16), I16)
    masks = din("masks", (128, 2 * ES), FP8)

    out_d = nc.dram_tensor("out", [NPAD, cfg.OUT], F32, kind="ExternalOutput")

    # dense-phase s-tile blocks per (chunk, core): batched loads/stores
    def blocks(ch):
        out, s = [], ch * CW
        end = (ch + 1) * CW
        while s < end:
            db = min(5, end - s)
            out.append((s, db))
            s += db
        return out

    with tile.TileContext(nc) as tc:
        with (
            tc.tile_pool(name="const", bufs=1) as cp,
            tc.tile_pool(name="gath", bufs=3) as gp,
            tc.tile_pool(name="mask", bufs=4) as mp,
            tc.tile_pool(name="work", bufs=3) as wp,
            tc.tile_pool(name="small", bufs=3) as sp2,
            tc.tile_pool(name="lt", bufs=3) as ltp,
            tc.tile_pool(name="stage", bufs=3) as sp,
            tc.tile_pool(name="psum", bufs=2, space="PSUM") as pp,
            tc.tile_pool(name="psd", bufs=2, space="PSUM") as pdp,
            tc.tile_pool(name="pst", bufs=2, space="PSUM") as ptp,
            tc.tile_pool(name="pald", bufs=2, space="PSUM") as pap,
            tc.tile_pool(name="dram", bufs=1, space="DRAM") as dp,
        ):
            ht = [dp.tile([NTOT, cfg.layers[l][3]], FP8, name=f"ht{l}",
                          tag=f"ht{l}") for l in range(3)]
            ag_in = [[dp.tile([2, 128, CW * 128], BF16,
                              name=f"agin{l}{h}", tag=f"agin{l}{h}")
                      for h in range(NCH)] for l in range(2)]
            # Shared tensors allow only one writing instruction -> per-rep
            ag_out_r = [[[dp.tile([NC, 2, 128, CW * 128], BF16,
                                  name=f"agout{r}{l}{h}", tag=f"agout{r}{l}{h}",
                                  addr_space="Shared")
                          for h in range(NCH)] for l in range(2)]
                        for r in range(repeat)]
            nc.gpsimd.load_library(mlp)

            def load_const(dram, shape, dtype=F32):
                t = cp.tile(list(shape), dtype, tag=dram.name, name=dram.name)
                nc.sync.dma_start(t[:], dram.ap())
                return t

            wcs = [load_const(wc[0], (128, cfg.F1 + 2 * cfg.HEADS), BF16),
                   load_const(wc[1], (128, 2 * (cfg.F1 + 2 * cfg.HEADS)), BF16),
                   load_const(wc[2], (128, 2 * (cfg.OUT + 2)), BF16)]
            idn = load_const(ident, (128, 128))
            brs = [load_const(brep[0], (128, cfg.F1)),
                   load_const(brep[1], (128, cfg.F1)),
                   load_const(brep[2], (128, cfg.OUT))]
            gih = load_const(gidx_d, (128, EI // 16), I16)

            for rep, l, (fin, fout, H, RW, relu) in [
                    (r, li, cfg.layers[li])
                    for r in range(repeat) for li in range(3)]:
                ag_out = ag_out_r[rep]
                KB = fin // 128
                NW2 = fout + 2 * H
                AOFF = fout                # byte offset of src logits (bf16)
                DOFF = fout + 2 * H        # byte offset of dst logits (bf16)

                # ---------- dense phase (last-processed chunks first: their
                # AllGathers finish first under descending edge ordering) ----
                for ch in range(NCH - 1, -1, -1) if "dense" not in abl else ():
                    for c in range(NC):
                        for s0, db in blocks(ch):
                            base = c * NPAD + s0 * 128
                            lt = ltp.tile([128, KB, db * 128], BF16, tag="lt")
                            if l == 0:
                                nc.sync.dma_start(
                                    lt[:], xT[:, base:base + db * 128]
                                    .unsqueeze(1))
                            else:
                                sb = s0 - ch * CW
                                src_ap = ag_out[l - 1][ch][
                                    c, :, :, sb * 128:(sb + db) * 128] \
                                    .rearrange("k p n -> p k n")
                                nc.sync.dma_start(lt[:], src_ap)
                            stg = sp.tile([128, db, RW], FP8, tag="hrow")
                            for i in range(db):
                                psd = pdp.tile([128, NW2], F32, tag="psd")
                                for kb in range(KB):
                                    nc.tensor.matmul(
                                        psd[:], lt[:, kb, i * 128:(i + 1) * 128],
                                        wcs[l][:, kb * NW2:(kb + 1) * NW2],
                                        start=(kb == 0), stop=(kb == KB - 1))
                                nc.scalar.activation(
                                    stg[:, i, 0:fout], psd[:, 0:fout],
                                    ACTF.Copy)
                                nc.scalar.activation(
                                    stg[:, i, AOFF:AOFF + 4 * H].bitcast(BF16),
                                    psd[:, fout:fout + 2 * H], ACTF.Copy)
                            nc.sync.dma_start(
                                ht[l][base:base + db * 128, :]
                                .rearrange("(d p) r -> p d r", p=128), stg[:])

                # ---------- edge phase: 5-stage software pipeline ----------
                # S0: loads + gathers; S1: dst-logit matmuls + exp; S15:
                # attention weights folded into features (DVE, the largest
                # elementwise op); S2: aggregation + output. Staggering
                # stages across windows keeps each engine's in-order queue
                # from serializing the cross-engine chain. Chunks run in
                # descending order; each chunk's AllGather starts as soon as
                # its windows are stored and overlaps the remaining edge
                # work, with the last AllGather covered by the next layer's
                # dense phase (which consumes chunks in the same order).
                worder = [w for ch in range(NCH - 1, -1, -1)
                          for w in range(ch * CW, (ch + 1) * CW)]
                st = {}

                def s0(w):
                    mk = mp.tile([128, 2, Tw, 128], FP8, tag="mk")
                    nc.sync.dma_start(
                        mk[:], masks[:, w * 2 * Tw * 128:(w + 1) * 2 * Tw * 128]
                        .rearrange("p (k t d) -> p k t d", k=2, d=128))
                    g1 = gp.tile([128, TG, RW], FP8, tag="g1")
                    for t0 in (range(0, TG, GCH) if "gather1" not in abl
                               else range(0, 1)):
                        ch = min(GCH, TG - t0)
                        isl = slice((w * TG + t0) * 8, (w * TG + t0 + ch) * 8)
                        nc.gpsimd.dma_gather(
                            g1[:, t0:t0 + ch, :], ht[l][:], gih[:, isl],
                            ch * 128, ch * 128, RW,
                            queue_num=(t0 // GCH) % NSWQ)
                    st[w] = {"mk": mk, "g1": g1}

                def s1(w):
                    g1, smT = st[w]["g1"], st[w]["mk"][:, 1]
                    # per-edge dst logits via transposed one-hot matmul
                    aldw = g1[:, 0, DOFF:DOFF + 2 * H].bitcast(BF16)
                    pald = pap.tile([128, Tw, H], F32, tag="pald")
                    for t in range(Tw if ("pald1" not in abl and "min" not in abl) else 1):
                        nc.tensor.matmul(pald[:, t, :], smT[:, t, :], aldw,
                                         start=True, stop=True)
                    als = g1[:, 1:Tw + 1, AOFF:AOFF + 2 * H].bitcast(BF16)
                    zb = sp2.tile([128, Tw, H], F32, tag="zb")
                    nc.vector.tensor_tensor(zb[:], als, pald[:], ALU.add)
                    zl = sp2.tile([128, Tw, H], F32, tag="zl")
                    nc.vector.scalar_tensor_tensor(
                        zl[:], zb[:], NEG_SLOPE, zb[:],
                        op0=ALU.mult, op1=ALU.max)
                    exw = sp2.tile([128, Tw, H], F32, tag="exw")
                    nc.scalar.activation(exw[:], zl[:], ACTF.Exp)
                    gx = wp.tile([128, Tw, fout + H], BF16, tag="gx")
                    nc.scalar.activation(gx[:, :, fout:fout + H], exw[:],
                                         ACTF.Copy)
                    # self-loop weight from the dst-extra tile (partition-
                    # aligned: lane d holds node d's own logits)
                    alss = g1[:, 0, AOFF:AOFF + 2 * H].bitcast(BF16)
                    zs = sp2.tile([128, H], F32, tag="zs")
                    nc.vector.tensor_tensor(zs[:], alss, aldw, ALU.add)
                    zsl = sp2.tile([128, H], F32, tag="zsl")
                    nc.vector.scalar_tensor_tensor(
                        zsl[:], zs[:], NEG_SLOPE, zs[:],
                        op0=ALU.mult, op1=ALU.max)
                    exs = sp2.tile([128, H], F32, tag="exs")
                    nc.scalar.activation(exs[:], zsl[:], ACTF.Exp)
                    st[w]["gx"] = gx
                    st[w]["exw"] = exw
                    st[w]["exs"] = exs

                def s15(w):
                    g1, exs = st[w]["g1"], st[w]["exs"]
                    # self-loop contribution ex_self * h_self, added into the
                    # aggregation PSUM in s2
                    sterm = sp2.tile([128, fout], F32, tag="sterm")
                    st4 = sterm[:, :].rearrange("p (h c) -> p h c", h=H)
                    hs4 = g1[:, 0, 0:fout].rearrange("p (h c) -> p h c", h=H)
                    exsb = exs[:, :].unsqueeze(2).broadcast_to(
                        (128, H, fout // H))
                    nc.vector.tensor_tensor(st4, hs4, exsb, ALU.mult)
                    st[w]["sterm"] = sterm
                    if "fold" in abl or "min" in abl:
                        return
                    gx, exw = st[w]["gx"], st[w]["exw"]
                    g1h = g1[:, 1:Tw + 1, 0:fout].rearrange(
                        "p t (h c) -> p t h c", h=H)
                    gx4 = gx[:, :, 0:fout].rearrange(
                        "p t (h c) -> p t h c", h=H)
                    exb = exw[:, :, :].unsqueeze(3).broadcast_to(
                        (128, Tw, H, fout // H))
                    nc.vector.tensor_tensor(gx4, g1h, exb, ALU.mult)

                def s2(w):
                    sm, gx = st[w]["mk"][:, 0], st[w]["gx"]
                    ps = pp.tile([128, fout + H], F32, tag="ps")
                    nagg = Tw if ("agg1" not in abl and "min" not in abl) else 1
                    for t in range(nagg):
                        nc.tensor.matmul(ps[:], sm[:, t, :],
                                         gx[:, t, 0:fout + H],
                                         start=(t == 0), stop=(t == nagg - 1))
                    # add the self-loop term (also keeps s > 0 for every dst,
                    # so no +EPS is needed)
                    nc.vector.tensor_tensor(ps[:, 0:fout], ps[:, 0:fout],
                                            st[w]["sterm"][:], ALU.add)
                    nc.vector.tensor_tensor(ps[:, fout:fout + H],
                                            ps[:, fout:fout + H],
                                            st[w]["exs"][:], ALU.add)
                    rs = sp2.tile([128, H], F32, tag="rs")
                    nc.vector.reciprocal(rs[:], ps[:, fout:fout + H])
                    ow = sp2.tile([128, fout], F32, tag="ow")
                    if H > 1:
                        u4 = ps[:, 0:fout].rearrange("p (h c) -> p h c", h=H)
                        o4 = ow[:, :].rearrange("p (h c) -> p h c", h=H)
                        rsb = rs[:, :].unsqueeze(2).broadcast_to(
                            (128, H, fout // H))
                        nc.vector.tensor_tensor(o4, u4, rsb, ALU.mult)
                    else:
                        nc.vector.tensor_scalar(
                            ow[:], ps[:, 0:fout], rs[:, 0:1], None, ALU.mult)
                    nc.vector.tensor_tensor(ow[:], ow[:], brs[l][:], ALU.add)
                    if relu:
                        nc.vector.tensor_scalar_max(ow[:], ow[:], 0.0)

                    if l < 2:
                        ch, wb = w // CW, w % CW
                        pst = ptp.tile([128, 2, 128], F32, tag="pst")
                        for hh in range(2):
                            nc.tensor.transpose(
                                pst[:, hh, :],
                                ow[:, hh * 128:(hh + 1) * 128], idn[:])
                        ts = sp.tile([128, 2, 128], BF16, tag="ts")
                        nc.vector.tensor_copy(ts[:], pst[:])
                        nc.sync.dma_start(
                            ag_in[l][ch][:, :, wb * 128:(wb + 1) * 128]
                            .rearrange("k p n -> p k n"), ts[:])
                    else:
                        nc.sync.dma_start(
                            out_d[w * 128:(w + 1) * 128, :], ow[:])
                    del st[w]

                def emit_cc(ch):
                    nc.gpsimd.collective_compute(
                        "AllGather", ALU.bypass,
                        replica_groups=[list(range(NC))],
                        ins=[ag_in[l][ch].opt()],
                        outs=[ag_out[l][ch].opt()])

                # emit chunk ch's AllGather 3 iterations after its last s2:
                # by then its wait (the chunk's stores) is satisfied when the
                # Pool sequencer reaches it, so queued gathers don't stall
                cc_at = {(NCH - q) * CW + CC_OFF: q for q in range(NCH)
                         if (NCH - q) * CW + CC_OFF <= NWIN + 2}

                for k in range(NWIN + 3) if "edge" not in abl else ():
                    if k < NWIN:
                        s0(worder[k])
                    if 1 <= k <= NWIN:
                        s1(worder[k - 1])
                    if 2 <= k <= NWIN + 1:
                        s15(worder[k - 2])
                    if k >= 3:
                        s2(worder[k - 3])
                    if l < 2 and "cc" not in abl and k in cc_at:
                        emit_cc(cc_at[k])

                if l < 2 and "cc" not in abl:
                    for q in range(NCH):
                        if (NCH - q) * CW + CC_OFF > NWIN + 2:
                            emit_cc(q)

    nc.compile()
    return nc


# ---------------------------------------------------------------------------

_HARD_CFG = Cfg()


def kernel_run(inputs, trace=False, trace_kwargs=None):
    cfg = _HARD_CFG
    x = np.asarray(inputs["x"])
    ei = np.asarray(inputs["edge_index"])
    shared, per_core, Tw = host_prep(cfg, x, ei, inputs)
    nc = build_module(cfg, Tw)
    in_maps = []
    for c in range(cfg.NC):
        m = dict(shared)
        m.update(per_core[c])
        in_maps.append(m)
    res = bass_utils.run_bass_kernel_spmd(
        nc, in_maps, core_ids=list(range(cfg.NC)), trace=trace,
        **(trace_kwargs or {}))
    out = np.concatenate(
        [res.results[c]["out"][:cfg.NLOC] for c in range(cfg.NC)], axis=0)
    return out.astype(np.float32), res


def kernel(**inputs):
    return kernel_run(inputs)[0]

